# revision 8
# baseline (speedup 1.0000x reference)
"""MoE genre-gate kernel for 8 Trainium2 NeuronCores.

Strategy (expert-parallel with token dispatch, per sharding hint):
  - Routing (RMSNorm -> word+genre gate -> softmax -> top-2) is computed on
    host in float64: it is 0.03% of the FLOPs and produces the data-dependent
    dispatch tables (the stand-in for all-to-all).
  - Token dispatch is a capacity-packed SPMD schedule: a small DP search
    finds per-slot capacities CS (e.g. [784, 292]) minimizing sum(CS)
    subject to packing each expert's tokens into 8 cells per slot column
    (cell = one expert, <= capacity).  Every core runs the same program:
    one slot of each capacity, receiving its cell's tokens (zero-padded).
  - Each core runs a dense 3-stage MLP over its slots in bfloat16 (full PE
    rate, FWL-hidden weight loads, ~4e-3 rel err vs the 2e-2 budget) with
    fp32 PSUM accumulation. With zero in-MLP biases (this problem), the host
    pre-scales each token row by its combine weight cw >= 0 (commutes with
    relu), so padding rows are exactly zero and stage 3 runs
    weight-stationary; a general biased fallback path is kept.
  - Stage 3 of slot s-1 is software-interleaved with stage 1 of slot s to
    keep the PE busy across stage boundaries (HAM clock-gate stays warm);
    weight loads are deduplicated via chunk-innermost loops + walrus
    ldw-opt.
  - Host scatter-adds the per-pair outputs back to [B,S,H] and adds the
    (cw @ b3) bias term.

Hardcoded problem shape: B=2, S=2048, H=1024, G=256, E=8, M=2048, top-2.
"""

import numpy as np

import concourse.bass as bass
import concourse.tile as tile
from concourse import mybir
from concourse.bass_utils import run_bass_kernel_spmd
import concourse.bass_utils as _bu

_orig_run_command = _bu.run_command


def _run_command_ldwopt(argv, **kwargs):
    argv = ["--enable-ldw-opt=true" if a == "--enable-ldw-opt=false" else a
            for a in argv]
    return _orig_run_command(argv, **kwargs)


TOP_K = 2
EPS = 1e-6
N_CORES = 8
H = 1024
M = 2048
KH, KM = H // 128, M // 128
F32R = mybir.dt.float32r
F32 = mybir.dt.float32
BF16 = mybir.dt.bfloat16
MM_DT = BF16          # matmul dtype (bf16: full PE rate, FWL hides ldweights)

if MM_DT == F32R:
    # fp32r matmuls self-load weights; walrus ldw-opt dedups repeated loads.
    # For bf16 the legalizer emits explicit InstLdweights, which ldw-opt
    # rejects ("InstLdweights is not compatible with LDW optimization").
    _bu.run_command = _run_command_ldwopt


# ---------------------------------------------------------------------------
# walrus in this container accepts only ONE sync-wait command per
# instruction; Tile emits up to ~10.  Split extras onto standalone NoOps on
# the same engine, inserted immediately before the instruction, which
# preserves per-engine program order and therefore semantics.
_ctr = [0]


def _legalize_waits(nc, max_waits=1):
    for f in nc.m.functions:
        for blk in f.blocks:
            out = []
            for inst in blk.instructions:
                si = inst.sync_info
                if si is not None and len(si.on_wait) > max_waits:
                    waits = list(si.on_wait)
                    extra, keep = waits[:-max_waits], waits[-max_waits:]
                    for w in extra:
                        _ctr[0] += 1
                        out.append(mybir.InstNoOp(
                            name=f"waitsplit-{_ctr[0]}",
                            engine=inst.engine, ins=[], outs=[],
                            sync_info=mybir.SyncInfo(on_wait=[w], on_update=[]),
                        ))
                    inst.sync_info = mybir.SyncInfo(
                        on_wait=keep, on_update=list(si.on_update))
                out.append(inst)
            blk.instructions = out


# ---------------------------------------------------------------------------
def _route(x2d, genre_embed, rms_w, wg_W, wg_b, gg_W, gg_b, B, S):
    """Host gating in float64. Returns combine weights [T, E] (zero outside
    top-2)."""
    xd = x2d.astype(np.float64)
    var = np.mean(xd * xd, axis=-1, keepdims=True)
    xn = rms_w.astype(np.float64) * (xd / np.sqrt(var + EPS))
    gate = xn @ wg_W.astype(np.float64) + wg_b.astype(np.float64)
    gg = genre_embed.astype(np.float64)[:, 0, :] @ gg_W.astype(np.float64) \
        + gg_b.astype(np.float64)                       # [B, E]
    gate = gate.reshape(B, S, -1) + gg[:, None, :]
    gate = gate.reshape(B * S, -1)
    gate -= gate.max(axis=-1, keepdims=True)
    p = np.exp(gate)
    p /= p.sum(axis=-1, keepdims=True)
    top2 = np.argsort(-p, axis=-1)[:, :TOP_K]
    cw = np.zeros_like(p)
    rows = np.arange(p.shape[0])[:, None]
    cw[rows, top2] = p[rows, top2]
    return cw.astype(np.float32)


# ---------------------------------------------------------------------------
def _pack_feasible(counts, CS):
    """Can each expert's counts be packed into 8 cells per slot column of
    capacity CS[j] (one expert per cell)?  Returns per-expert cell
    allocation [(a_0..a_{n-1})] or None."""
    n = len(CS)
    states = {tuple([0] * n): []}          # used-cells -> per-expert allocs
    for c in counts:
        new = {}
        for st, hist in states.items():
            avail = [N_CORES - st[j] for j in range(n)]

            def rec(j, rem, alloc):
                if rem <= 0:
                    key = tuple(st[k] + alloc[k] for k in range(n))
                    if key not in new:
                        new[key] = hist + [tuple(alloc)]
                    return
                if j == n:
                    return
                for a in range(avail[j] + 1):
                    rec(j + 1, rem - a * CS[j], alloc[:j] + [a] + alloc[j + 1:])
            rec(0, c, [0] * n)
        # prune dominated states
        keys = sorted(new)
        pruned = {}
        for k in keys:
            if not any(all(p[i] <= k[i] for i in range(n)) and p != k
                       for p in pruned):
                pruned[k] = new[k]
        states = pruned
        if not states:
            return None
    return next(iter(states.values()))


def _partition(counts):
    """Find 2-slot capacities CS (sum minimized via DP packing search) and
    the per-core cell assignment.

    Returns (CS, slots): slots[core][j] = (expert, lo, hi) token range."""
    lo_bound = max(2, -(-sum(counts) // N_CORES) // 2)
    best = None
    for cs0 in range(max(512, lo_bound), 1026, 2):
        if best is not None and cs0 >= best[0]:
            break
        lo, hi = 2, cs0
        found = None
        while lo <= hi:
            mid = (lo + hi) // 2
            mid -= mid % 2
            if mid < 2:
                mid = 2
            if best is not None and cs0 + mid >= best[0]:
                hi = mid - 2
                continue
            alloc = _pack_feasible(counts, [cs0, mid])
            if alloc is not None:
                found = (mid, alloc)
                hi = mid - 2
            else:
                lo = mid + 2
        if found is not None and (best is None or cs0 + found[0] < best[0]):
            best = (cs0 + found[0], [cs0, found[0]], found[1])
    if best is None:                      # fallback: single class of max count
        cmax = max(counts)
        cmax += cmax % 2
        best = (2 * cmax, [cmax, cmax],
                [(1, 1) if c else (0, 0) for c in counts])
    CS, alloc = best[1], best[2]

    # build cells: per column j, list of (expert, lo, hi)
    cols = [[] for _ in CS]
    for e, a in enumerate(alloc):
        c, pos = counts[e], 0
        # fill column 0 cells first (larger), remainder spread into later cols
        for j in range(len(CS)):
            for _ in range(a[j]):
                take = min(CS[j], c - pos)
                cols[j].append((e, pos, pos + take))
                pos += take
        assert pos == c, (e, pos, c, a)
    # pad columns with empty cells, one cell of each column per core
    slots = []
    for core in range(N_CORES):
        row = []
        for j in range(len(CS)):
            row.append(cols[j][core] if core < len(cols[j]) else (0, 0, 0))
        slots.append(row)
    return CS, slots


def _token_chunks(C):
    """Split C into matmul moving-dim chunks, each <=512 (PSUM bank) and as
    equal as possible."""
    n = -(-C // 512)
    base, rem = divmod(C, n)
    sizes = [base + (1 if i < rem else 0) for i in range(n)]
    assert sum(sizes) == C and all(s <= 512 for s in sizes)
    return sizes


# ---------------------------------------------------------------------------
def _build_program(CS, prescaled=False, legalize=True):
    """Emit the SPMD Bass program; CS = per-slot-class capacities.

    prescaled=True (valid when b1==b2==0): host pre-scales x rows by cw
    (cw>=0 commutes with relu), so no bias/cw tiles are needed and stage 3
    runs weight-stationary with output layout [H, C] (y transposed)."""
    S = len(CS)
    nc = bass.Bass()
    xt_d = [nc.dram_tensor(f"XT{s}", [H, CS[s]], MM_DT, kind="ExternalInput") for s in range(S)]
    w1_d = [nc.dram_tensor(f"W1{s}", [KM, 128, H], MM_DT, kind="ExternalInput") for s in range(S)]
    w2_d = [nc.dram_tensor(f"W2{s}", [KM, 128, M], MM_DT, kind="ExternalInput") for s in range(S)]
    if prescaled:
        w3_d = [nc.dram_tensor(f"W3{s}", [KH, 128, M], MM_DT, kind="ExternalInput") for s in range(S)]
        y_d = [nc.dram_tensor(f"Y{s}", [H, CS[s]], F32, kind="ExternalOutput") for s in range(S)]
    else:
        w3_d = [nc.dram_tensor(f"W3{s}", [M, H], MM_DT, kind="ExternalInput") for s in range(S)]
        y_d = [nc.dram_tensor(f"Y{s}", [CS[s], H], F32, kind="ExternalOutput") for s in range(S)]
        b1_d = [nc.dram_tensor(f"B1{s}", [M], F32, kind="ExternalInput") for s in range(S)]
        b2_d = [nc.dram_tensor(f"B2{s}", [M], F32, kind="ExternalInput") for s in range(S)]
        cw_d = [nc.dram_tensor(f"CW{s}", [CS[s]], F32, kind="ExternalInput") for s in range(S)]

    HB = 512 if len(CS) >= 3 else 256

    with tile.TileContext(nc) as tc:
        with (
            tc.tile_pool(name="xt", bufs=1) as p_xt,
            tc.tile_pool(name="w1", bufs=6) as p_w1,
            tc.tile_pool(name="w2", bufs=8) as p_w2,
            tc.tile_pool(name="w3", bufs=(5 if prescaled else 8)) as p_w3,
            tc.tile_pool(name="h1", bufs=1) as p_h1,
            tc.tile_pool(name="h2", bufs=1) as p_h2,
            tc.tile_pool(name="bias", bufs=1) as p_b,
            tc.tile_pool(name="y", bufs=4) as p_y,
            tc.tile_pool(name="ps", bufs=8, space="PSUM") as p_ps,
        ):
            st = [dict() for _ in range(S)]   # per-slot tiles/geometry

            def emit_loads(s):
                C = CS[s]
                v = st[s]
                v["tcs"] = _token_chunks(C)
                v["tco"] = np.cumsum([0] + v["tcs"]).tolist()
                v["tts"] = [(i * 128, min(128, C - i * 128)) for i in range(-(-C // 128))]
                v["xt"] = [p_xt.tile([128, C], MM_DT, tag=f"xt{k}", name=f"xt_{s}_{k}") for k in range(KH)]
                for k in range(KH):
                    nc.sync.dma_start(out=v["xt"][k][:], in_=xt_d[s][k * 128:(k + 1) * 128, :])
                if prescaled:
                    v["h1"] = [p_h1.tile([128, C], MM_DT, tag=f"h1_{m}", name=f"h1_{s}_{m}") for m in range(KM)]
                    v["h2"] = [p_h2.tile([128, C], MM_DT, tag=f"h2_{m}", name=f"h2_{s}_{m}") for m in range(KM)]
                    return
                v["b1t"] = [p_b.tile([128, 1], F32, tag=f"b1_{s}_{m}", name=f"b1t_{s}_{m}") for m in range(KM)]
                v["b2t"] = [p_b.tile([128, 1], F32, tag=f"b2_{s}_{m}", name=f"b2t_{s}_{m}") for m in range(KM)]
                for m in range(KM):
                    nc.scalar.dma_start(out=v["b1t"][m][:], in_=b1_d[s][m * 128:(m + 1) * 128].rearrange("(p one) -> p one", one=1))
                    nc.scalar.dma_start(out=v["b2t"][m][:], in_=b2_d[s][m * 128:(m + 1) * 128].rearrange("(p one) -> p one", one=1))
                v["cwt"] = []
                for t, (t0, tn) in enumerate(v["tts"]):
                    v["cwt"].append(p_b.tile([tn, 1], F32, tag=f"cw_{s}_{t}", name=f"cwt_{s}_{t}"))
                    nc.scalar.dma_start(out=v["cwt"][t][:], in_=cw_d[s][t0:t0 + tn].rearrange("(p one) -> p one", one=1))
                v["h1"] = [p_h1.tile([128, C], MM_DT, tag=f"h1_{m}", name=f"h1_{s}_{m}") for m in range(KM)]
                v["h2"] = [p_h2.tile([128, C], MM_DT, tag=f"h2_{m}", name=f"h2_{s}_{m}") for m in range(KM)]

            def st1_group(s, m):
                v = st[s]
                w1t = p_w1.tile([128, H], MM_DT, tag="w1", name=f"w1t_{s}_{m}")
                nc.gpsimd.dma_start(out=w1t[:], in_=w1_d[s][m])
                pss = [p_ps.tile([128, tcz], F32, tag="ps", name=f"ps1_{s}_{m}_{ci}")
                       for ci, tcz in enumerate(v["tcs"])]
                for k in range(KH):
                    for ci in range(len(v["tcs"])):
                        nc.tensor.matmul(
                            pss[ci][:], w1t[:, k * 128:(k + 1) * 128],
                            v["xt"][k][:, v["tco"][ci]:v["tco"][ci + 1]],
                            start=(k == 0), stop=(k == KH - 1))
                for ci in range(len(v["tcs"])):
                    nc.scalar.activation(
                        v["h1"][m][:, v["tco"][ci]:v["tco"][ci + 1]], pss[ci][:],
                        mybir.ActivationFunctionType.Relu,
                        **({} if prescaled else {"bias": v["b1t"][m][:, 0:1]}))

            def st1_batch(s, ms):
                """k-outer stage-1 over a batch of m-groups: the first matmul
                needs only xt[0] + this batch's w1 tiles, so slot-0 compute
                starts ~4us earlier than m-outer (which needs all xt tiles).
                Batch PSUM footprint: len(ms) * n_chunks banks (<= 8)."""
                v = st[s]
                nch = len(v["tcs"])
                w1ts, pss = {}, {}
                for m in ms:
                    w1ts[m] = p_w1.tile([128, H], MM_DT, tag="w1", name=f"w1t_{s}_{m}")
                    nc.gpsimd.dma_start(out=w1ts[m][:], in_=w1_d[s][m])
                    for ci, tcz in enumerate(v["tcs"]):
                        pss[m, ci] = p_ps.tile([128, tcz], F32, tag="ps",
                                               name=f"ps1_{s}_{m}_{ci}")
                for k in range(KH):
                    for m in ms:
                        for ci in range(nch):
                            nc.tensor.matmul(
                                pss[m, ci][:], w1ts[m][:, k * 128:(k + 1) * 128],
                                v["xt"][k][:, v["tco"][ci]:v["tco"][ci + 1]],
                                start=(k == 0), stop=(k == KH - 1))
                for m in ms:
                    for ci in range(nch):
                        nc.scalar.activation(
                            v["h1"][m][:, v["tco"][ci]:v["tco"][ci + 1]],
                            pss[m, ci][:], mybir.ActivationFunctionType.Relu,
                            **({} if prescaled else {"bias": v["b1t"][m][:, 0:1]}))

            def st2_group(s, m):
                v = st[s]
                w2t = p_w2.tile([128, M], MM_DT, tag="w2", name=f"w2t_{s}_{m}")
                # w2 rides the sync DMA queue so the slot's full 16.8MB
                # weight stream is split across two queues; a small slot
                # (C~290) otherwise starves the PE on one queue.
                nc.sync.dma_start(out=w2t[:], in_=w2_d[s][m])
                pss = [p_ps.tile([128, tcz], F32, tag="ps", name=f"ps2_{s}_{m}_{ci}")
                       for ci, tcz in enumerate(v["tcs"])]
                for k in range(KM):
                    for ci in range(len(v["tcs"])):
                        nc.tensor.matmul(
                            pss[ci][:], w2t[:, k * 128:(k + 1) * 128],
                            v["h1"][k][:, v["tco"][ci]:v["tco"][ci + 1]],
                            start=(k == 0), stop=(k == KM - 1))
                for ci in range(len(v["tcs"])):
                    nc.scalar.activation(
                        v["h2"][m][:, v["tco"][ci]:v["tco"][ci + 1]], pss[ci][:],
                        mybir.ActivationFunctionType.Relu,
                        **({} if prescaled else {"bias": v["b2t"][m][:, 0:1]}))

            def emit_w3(s, hb):
                w3t = [p_w3.tile([128, HB], MM_DT, tag=f"w3_{k % 4}", name=f"w3t_{s}_{hb}_{k}") for k in range(KM)]
                for k in range(KM):
                    nc.gpsimd.dma_start(
                        out=w3t[k][:],
                        in_=w3_d[s][k * 128:(k + 1) * 128, hb * HB:(hb + 1) * HB])
                st[s][f"w3_{hb}"] = w3t

            def st3_group(s, hb, t):
                v = st[s]
                t0, tn = v["tts"][t]
                w3t = v[f"w3_{hb}"]
                ps = p_ps.tile([tn, HB], F32, tag="ps", name=f"ps3_{s}_{hb}_{t}")
                for k in range(KM):
                    nc.tensor.matmul(
                        ps[:], v["h2"][k][:, t0:t0 + tn], w3t[k][:],
                        start=(k == 0), stop=(k == KM - 1))
                yt = p_y.tile([tn, HB], F32, tag="y", name=f"yt_{s}_{hb}_{t}")
                nc.scalar.activation(
                    yt[:], ps[:], mybir.ActivationFunctionType.Copy,
                    scale=v["cwt"][t][:, 0:1])
                nc.scalar.dma_start(
                    out=y_d[s][t0:t0 + tn, hb * HB:(hb + 1) * HB],
                    in_=yt[:])

            def st3_group_ws(s, hm):
                """Weight-stationary stage 3 (prescaled mode): out y_T[h, tok]."""
                v = st[s]
                w3t = p_w3.tile([128, M], MM_DT, tag="w3ws", name=f"w3ws_{s}_{hm}")
                nc.gpsimd.dma_start(out=w3t[:], in_=w3_d[s][hm])
                pss = [p_ps.tile([128, tcz], F32, tag="ps", name=f"ps3_{s}_{hm}_{ci}")
                       for ci, tcz in enumerate(v["tcs"])]
                for k in range(KM):
                    for ci in range(len(v["tcs"])):
                        nc.tensor.matmul(
                            pss[ci][:], w3t[:, k * 128:(k + 1) * 128],
                            v["h2"][k][:, v["tco"][ci]:v["tco"][ci + 1]],
                            start=(k == 0), stop=(k == KM - 1))
                for ci, tcz in enumerate(v["tcs"]):
                    yt = p_y.tile([128, tcz], F32, tag="y", name=f"yt_{s}_{hm}_{ci}")
                    nc.scalar.activation(
                        yt[:], pss[ci][:], mybir.ActivationFunctionType.Copy)
                    nc.scalar.dma_start(
                        out=y_d[s][hm * 128:(hm + 1) * 128, v["tco"][ci]:v["tco"][ci + 1]],
                        in_=yt[:])

            def st3_emitters(s):
                if prescaled:
                    return [lambda s=s, hm=hm: st3_group_ws(s, hm) for hm in range(KH)]
                ems = []
                for hb in range(H // HB):
                    if hb > 0:
                        ems.append(lambda s=s, hb=hb: emit_w3(s, hb))
                    for t in range(len(st[s]["tts"])):
                        ems.append(lambda s=s, hb=hb, t=t: st3_group(s, hb, t))
                return ems

            def interleave(a_ems, b_ems):
                """Emit a and b emitter lists merged evenly (b spread among a)."""
                na, nb = len(a_ems), len(b_ems)
                bi = 0
                for i, a in enumerate(a_ems):
                    while bi < nb and bi * na <= i * nb:
                        b_ems[bi]()
                        bi += 1
                    a()
                while bi < nb:
                    b_ems[bi]()
                    bi += 1

            # ---- PE pre-warm: a short junk burst bridges the gap between
            # the framework preamble (~6.3us) and the first real matmul
            # (slot-0 stage 1 starts once xt[0] + 2 w1 tiles land) ----
            junk = p_b.tile([128, 512], F32, tag="warm", name="warm_src")
            nc.gpsimd.memset(junk[:], 0.0)
            psw = p_ps.tile([128, 512], F32, tag="ps", name="warm_ps")
            for i in range(4):
                nc.tensor.matmul(psw[:], junk[:, 0:128], junk[:],
                                 start=(i == 0), stop=(i == 3))

            def st1_batches(s):
                """Batch sizes: small first batches so the first matmuls wait
                on as little weight DMA as possible; PSUM cap 8 banks."""
                nch = len(_token_chunks(CS[s]))
                bsz = max(1, 8 // nch // 2)
                first = [bsz, bsz] if 2 * bsz <= KM else [bsz]
                rest_each = max(1, 8 // nch)
                rem = KM - sum(first)
                sizes = list(first)
                while rem > 0:
                    take = min(rest_each, rem)
                    sizes.append(take)
                    rem -= take
                ms, out = 0, []
                for sz in sizes:
                    out.append(list(range(ms, ms + sz)))
                    ms += sz
                return out

            # ---- emission schedule: st3(s-1) interleaves with st1(s) ----
            emit_loads(0)
            prev_st3 = []
            for s in range(S):
                if s > 0:
                    emit_loads(s)
                if s == 0:
                    for ms in st1_batches(0):
                        st1_batch(0, ms)
                else:
                    interleave([lambda s=s, m=m: st1_group(s, m) for m in range(KM)],
                               prev_st3)
                if not prescaled:
                    emit_w3(s, 0)    # prefetch stage-3 hb=0 weights early
                for m in range(KM):
                    st2_group(s, m)
                prev_st3 = st3_emitters(s)
            for em in prev_st3:
                em()

    if legalize:
        _legalize_waits(nc)
    return nc


# ---------------------------------------------------------------------------
def _run_spmd(CS, prescaled, in_maps):
    """Compile + run on cores 0-7. On a transient device failure (e.g.
    NRT_EXEC_UNIT_UNRECOVERABLE from a stale runtime state), retry in a
    fresh subprocess whose NRT session starts clean."""
    try:
        nc = _build_program(CS, prescaled=prescaled)
        return run_bass_kernel_spmd(nc, in_maps, list(range(N_CORES))).results
    except Exception:
        import os
        import pickle
        import subprocess
        import sys
        import tempfile
        d = tempfile.mkdtemp()
        inp, outp = os.path.join(d, "in.pkl"), os.path.join(d, "out.pkl")
        with open(inp, "wb") as f:
            pickle.dump((CS, prescaled, in_maps), f)
        code = (
            "import pickle, sys\n"
            f"sys.path.insert(0, {os.path.dirname(os.path.abspath(__file__))!r})\n"
            "import kernel as K\n"
            f"CS, prescaled, in_maps = pickle.load(open({inp!r}, 'rb'))\n"
            "nc = K._build_program(CS, prescaled=prescaled)\n"
            "from concourse.bass_utils import run_bass_kernel_spmd\n"
            "r = run_bass_kernel_spmd(nc, in_maps, list(range(K.N_CORES))).results\n"
            f"pickle.dump(r, open({outp!r}, 'wb'))\n"
        )
        err = None
        for _ in range(2):
            try:
                subprocess.run([sys.executable, "-c", code], check=True,
                               timeout=1800)
                with open(outp, "rb") as f:
                    return pickle.load(f)
            except Exception as e:
                err = e
        raise err


def kernel(x, genre_embed, rms_w, wg_W, wg_b, gg_W, gg_b, W1, b1, W2, b2, W3, b3):
    x = np.asarray(x, np.float32)
    B, S_, _ = x.shape
    T = B * S_
    x2d = np.ascontiguousarray(x.reshape(T, H))
    W1 = np.asarray(W1, np.float32)
    W2 = np.asarray(W2, np.float32)
    W3 = np.asarray(W3, np.float32)

    if MM_DT == BF16:
        import ml_dtypes
        host_dt = ml_dtypes.bfloat16
    else:
        host_dt = np.float32

    cw = _route(x2d, np.asarray(genre_embed, np.float32), np.asarray(rms_w, np.float32),
                np.asarray(wg_W, np.float32), np.asarray(wg_b, np.float32),
                np.asarray(gg_W, np.float32), np.asarray(gg_b, np.float32), B, S_)
    E = cw.shape[1]
    tok_by_e = [np.nonzero(cw[:, e])[0] for e in range(E)]
    counts = [len(t) for t in tok_by_e]
    CS, slots = _partition(counts)

    # prescaled mode is exact when the in-MLP biases are zero (cw >= 0
    # commutes with relu); b3 is always applied on the host via cw @ b3
    prescaled = not (np.any(np.asarray(b1)) or np.any(np.asarray(b2)))

    # pre-tile weights once per expert (shared across cores)
    used = set(e for core in slots for (e, lo, hi) in core if hi > lo)
    w1_tiled, w2_tiled, w3_tiled = {}, {}, {}
    for e in used:
        w1_tiled[e] = np.ascontiguousarray(
            W1[e].reshape(KH, 128, KM, 128).transpose(2, 1, 0, 3).reshape(KM, 128, H)).astype(host_dt)
        w2_tiled[e] = np.ascontiguousarray(
            W2[e].reshape(KM, 128, KM, 128).transpose(2, 1, 0, 3).reshape(KM, 128, M)).astype(host_dt)
        if prescaled:
            w3_tiled[e] = np.ascontiguousarray(
                W3[e].reshape(KM, 128, KH, 128).transpose(2, 1, 0, 3).reshape(KH, 128, M)).astype(host_dt)
        else:
            w3_tiled[e] = W3[e].astype(host_dt)
    z1 = np.zeros((KM, 128, H), host_dt)
    z2 = np.zeros((KM, 128, M), host_dt)
    z3 = np.zeros((KH, 128, M), host_dt) if prescaled else np.zeros((M, H), host_dt)

    in_maps = []
    meta = []
    for core in range(N_CORES):
        im = {}
        cmeta = []
        for si, (e, lo, hi) in enumerate(slots[core]):
            C = CS[si]
            idx = tok_by_e[e][lo:hi] if hi > lo else np.zeros((0,), np.int64)
            n = len(idx)
            xt = np.zeros((H, C), host_dt)
            if prescaled:
                xt[:, :n] = (x2d[idx] * cw[idx, e][:, None]).T.astype(host_dt)
            else:
                xt[:, :n] = x2d[idx].T.astype(host_dt)
            im[f"XT{si}"] = xt
            im[f"W1{si}"] = w1_tiled[e] if n else z1
            im[f"W2{si}"] = w2_tiled[e] if n else z2
            im[f"W3{si}"] = w3_tiled[e] if n else z3
            if not prescaled:
                cwc = np.zeros((C,), np.float32)
                cwc[:n] = cw[idx, e]
                im[f"B1{si}"] = np.asarray(b1[e], np.float32)
                im[f"B2{si}"] = np.asarray(b2[e], np.float32)
                im[f"CW{si}"] = cwc
            cmeta.append(idx)
        in_maps.append(im)
        meta.append(cmeta)

    results = _run_spmd(CS, prescaled, in_maps)

    out2d = cw @ np.asarray(b3, np.float32)      # bias-3 combine term [T, H]
    for core in range(N_CORES):
        for si, idx in enumerate(meta[core]):
            if not len(idx):
                continue
            y = results[core][f"Y{si}"]
            if prescaled:
                out2d[idx] += y[:, :len(idx)].T
            else:
                out2d[idx] += y[:len(idx)]
    return out2d.reshape(B, S_, H).astype(np.float32)
